# revision 64
# baseline (speedup 1.0000x reference)
"""Trainium2 Bass kernel for the Calibrated Spectral Mixer (two-domain).

Math (per batch b):
    fx   = x @ W_in + b_in                      [N, H*DH]
    spec_h = inv_in_h^T @ fx_h                  [G, DH]   (contract over N)
    spec = LN_{(G,DH)}(spec) * ln_g + ln_b
    spec2_h = spec_h @ mlp_w
    out_h = inv_out_h @ spec2_h                 [N, DH]
    y    = concat_h(out_h) @ W_out + b_out      [N, C]

Device algebra (folded):
    E^T   = x^T-contraction:  E^T[c,g] = sum_n x[n,c] inv_in[n,g]
    specT_h[dh,g] = W_in_h^T @ E^T (+ b_in_h ⊗ s_in, s_in = sum_n inv_in[:,g])
    LN applied on packed specT, then spec2T_h = mlp_w^T @ affineT_h
    D[g,c] = sum_h spec2_h @ W_out_h    (heads summed when basis shared)
    yT[c,n] = D^T-contraction with inv_out^T  (+ b_out per-partition)

Sharding: data-parallel over B; core i computes batch i. No collectives.
If inv_in / inv_out are head-broadcast (they are, for the reference input
generator), only one head's basis is shipped/read (1 MB instead of 8 MB each).
"""

import os
import sys

sys.path.insert(0, "/opt/trn_rl_repo")

import numpy as np

import concourse.bass as bass
import concourse.tile as tile
from concourse import bacc, mybir
from concourse.bass_utils import run_bass_kernel_spmd

B, N, C, H, DH, G = 8, 8192, 128, 8, 64, 32
HD = H * DH  # 512
NT = N // 128  # 64 n-tiles of 128 rows
LN_EPS = 1e-5
F32 = mybir.dt.float32
F16 = mybir.dt.float16
BF16 = mybir.dt.bfloat16

_cache: dict = {}
LAST_EXEC_NS = None


def _ind2t():
    a = np.zeros((2, 128), np.float32)
    a[0, 0:64] = 1.0
    a[1, 64:128] = 1.0
    return a


def _mw_pack(mlp_w, W_out, shared):
    mw = np.einsum(
        "io,hoc->hic",
        np.asarray(mlp_w, np.float32),
        np.asarray(W_out, np.float32).reshape(H, DH, C),
    )  # [H, DH, C]
    if shared:
        out = np.empty((128, 4 * C), np.float32)
        for q in range(4):
            out[0:DH, q * C : (q + 1) * C] = mw[2 * q]
            out[DH:128, q * C : (q + 1) * C] = mw[2 * q + 1]
    else:
        out = np.zeros((128, H * C), np.float32)
        for h in range(H):
            b = (h % 2) * DH
            out[b : b + DH, h * C : (h + 1) * C] = mw[h]
    return np.ascontiguousarray(out)


def _build(shared: bool, with_bias: bool):
    """Build + compile the per-core Bass program.

    shared=True: inv bases identical across heads (1-head basis shipped).
    with_bias=False: b_in is all zeros -> skip the s_in / correction matmuls.
    """
    nc = bacc.Bacc("TRN2", target_bir_lowering=False, debug=False)

    # x and the bases are shipped as fp16 hi/lo pairs (same total bytes as
    # fp32); fp16 hi+lo carries ~22 mantissa bits, and the 3-term product
    # xh*ih + xh*il + xl*ih runs on the PE at 1 cy/col instead of fp32's
    # two-pass half-speed emulation.
    # xhl: host-pre-shuffled [p, (t, hi/lo, c)] so every DMA is 2D with long
    # contiguous runs per partition (cheap HWDGE descriptor generation)
    xhl_d = nc.dram_tensor("xhl", [128, NT * 2 * C], F16, kind="ExternalInput")
    w_in_d = nc.dram_tensor("w_in", [C, HD], F32, kind="ExternalInput")
    mw_d = nc.dram_tensor("mw", [128, (4 if shared else 8) * C], F32, kind="ExternalInput")
    ind2t_d = nc.dram_tensor("ind2t", [2, 128], F32, kind="ExternalInput")
    lngt_d = nc.dram_tensor("lngt", [128, 128], F32, kind="ExternalInput")
    lnbt_d = nc.dram_tensor("lnbt", [128, 128], F32, kind="ExternalInput")
    b_in_d = nc.dram_tensor("b_in", [1, HD], F32, kind="ExternalInput")
    b_out_d = nc.dram_tensor("b_out", [C, 1], F32, kind="ExternalInput")
    if shared:
        # ihl: host-pre-shuffled [p, (s, hi/lo, g)]
        ihl_d = nc.dram_tensor("ihl", [128, NT * 2 * G], F16, kind="ExternalInput")
        # stacked decode basis: 4 blocks of [Th; Tl] (duplicated on device)
        invtst_d = nc.dram_tensor("invtst", [4 * 64, 2048], BF16, kind="ExternalInput")
    else:
        # ihl: host-pre-shuffled [p, (h, s, hi/lo, g)]
        ihl_d = nc.dram_tensor("ihl", [128, H * NT * 2 * G], F16, kind="ExternalInput")
        invth_d = nc.dram_tensor("invth", [H * G, N], BF16, kind="ExternalInput")
        invtl_d = nc.dram_tensor("invtl", [H * G, N], BF16, kind="ExternalInput")
    yt_d = nc.dram_tensor("yT", [C, N], F32, kind="ExternalOutput")

    with tile.TileContext(nc) as tc:
        with (
            tc.tile_pool(name="res", bufs=1) as res,
            tc.tile_pool(name="work", bufs=1) as work,
            tc.tile_pool(name="yout", bufs=3) as yout,
            tc.tile_pool(name="ps", bufs=1, space="PSUM") as ps,
            tc.tile_pool(name="psy", bufs=4 if shared else 2, space="PSUM") as psy,
        ):
            # ---- resident loads ----
            # All DMAs are plain 2D copies (host pre-shuffled the layouts).
            # Issue is spread over both HWDGE rings (sync + scalar) and SWDGE
            # (gpsimd) so descriptor generation isn't serialized on one
            # sequencer; per-engine program order = need order.
            iw = NT * 2 * G if shared else H * NT * 2 * G
            ihl_sb = res.tile([128, iw], F16, tag="ihl")
            if shared:
                for k in range(4):
                    eng = nc.sync if k % 2 == 0 else nc.scalar
                    eng.dma_start(
                        out=ihl_sb[:, k * 1024 : (k + 1) * 1024],
                        in_=ihl_d[:, k * 1024 : (k + 1) * 1024],
                    )
            else:
                half = iw // 2
                nc.sync.dma_start(out=ihl_sb[:, 0:half], in_=ihl_d[:, 0:half])
                nc.scalar.dma_start(out=ihl_sb[:, half:iw], in_=ihl_d[:, half:iw])

            # x: slab 0 split into 4 fine tiles (encode can start after
            # ~256 KB lands); slabs 1-3 are 1 MB tiles with 2 DMAs each
            xtiles = []   # (tile, tiles_per, t0)
            for j in range(4):
                xt = res.tile([128, 4 * 2 * C], F16, tag=f"x0{j}")
                eng = nc.sync if j % 2 == 0 else nc.scalar
                eng.dma_start(
                    out=xt[:], in_=xhl_d[:, j * 1024 : (j + 1) * 1024]
                )
                xtiles.append((xt, 4, 4 * j))
            for i in range(2, 8):
                xt = res.tile([128, 8 * 2 * C], F16, tag=f"x{i}")
                eng = nc.sync if i % 2 == 0 else nc.scalar
                eng.dma_start(
                    out=xt[:], in_=xhl_d[:, i * 2048 : (i + 1) * 2048]
                )
                xtiles.append((xt, 8, 8 * i))

            if shared:
                pass
            else:
                # invT hi/lo [H*G, N] -> [128, (j, kb, blk, f)]: partition = hg%128
                invt_sb = res.tile([128, 2 * 2 * 4 * 2048], BF16, tag="invt")
                for j, src in enumerate((invth_d, invtl_d)):
                    for kb in range(2):
                        for blk in range(4):
                            nc.gpsimd.dma_start(
                                out=invt_sb[:, j * 16384 + kb * 8192 + blk * 2048 :][:, 0:2048],
                                in_=src[kb * 128 : (kb + 1) * 128,
                                        blk * 2048 : (blk + 1) * 2048],
                            )

            w_in_sb = res.tile([C, HD], F32, tag="w_in")
            nc.gpsimd.dma_start(out=w_in_sb[:], in_=w_in_d[:])
            # mw = mlp_w @ W_out_h, host-precomputed (head-pair stacked)
            mw_sb = res.tile([128, (4 if shared else 8) * C], F32, tag="mw")
            nc.gpsimd.dma_start(out=mw_sb[:], in_=mw_d[:])
            lngt_sb = res.tile([128, 128], F32, tag="lngt")
            nc.gpsimd.dma_start(out=lngt_sb[:], in_=lngt_d[:])
            lnbt_sb = res.tile([128, 128], F32, tag="lnbt")
            nc.gpsimd.dma_start(out=lnbt_sb[:], in_=lnbt_d[:])
            b_in_sb = res.tile([1, HD], F32, tag="b_in")
            nc.gpsimd.dma_start(out=b_in_sb[:], in_=b_in_d[:])
            b_out_sb = res.tile([C, 1], F32, tag="b_out")
            nc.gpsimd.dma_start(out=b_out_sb[:], in_=b_out_d[:])

            if shared:
                invt_half = []
                for v in range(2):
                    th = res.tile([128, 2048], BF16, tag=f"invth{v}")
                    nc.gpsimd.dma_start(
                        out=th[:], in_=invtst_d[v * 128 : (v + 1) * 128, :]
                    )
                    invt_half.append(th)
                # duplicate [Th;Tl] -> [Th;Tl;Th;Tl] with idle-DVE copies
                invt_tiles = []
                for u in range(4):
                    tt = res.tile([128, 2048], BF16, tag=f"invt{u}")
                    src = invt_half[u // 2][(u % 2) * 64 : (u % 2) * 64 + 64, :]
                    nc.vector.tensor_copy(tt[0:64, :], src)
                    nc.vector.tensor_copy(tt[64:128, :], src)
                    invt_tiles.append(tt)

            ones16 = res.tile([128, 1], F16, tag="ones16")
            nc.vector.memset(ones16[:], 1.0)
            # half-indicators, pre-scaled so the stats matmul yields means
            ind2 = res.tile([128, 2], F32, tag="ind2")
            nc.vector.memset(ind2[:], 0.0)
            nc.vector.memset(ind2[0:64, 0:1], 1.0 / (G * DH))
            nc.vector.memset(ind2[64:128, 1:2], 1.0 / (G * DH))
            ind2t = res.tile([2, 128], F32, tag="ind2t")
            nc.gpsimd.dma_start(out=ind2t[:], in_=ind2t_d[:])
            eps_sb = res.tile([2, 1], F32, tag="eps")
            nc.vector.memset(eps_sb[:], LN_EPS)

            # PE warm-up: the HAM clock-gate only ramps to 2.4 GHz after
            # ~3.4 us of sustained activity and decays when PE idles. Dummy
            # matmuls (result discarded) run during DMA stalls and mid-phase
            # gaps so the real matmuls execute warm.
            def warm(n):
                wp = psy.tile([1, 512], F32, tag="y")
                for _ in range(n):
                    nc.tensor.matmul(
                        wp[0:1, 0:512],
                        ones16[:],
                        ihl_sb[:, 0:512],
                        start=True,
                        stop=True,
                    )

            warm(12)

            # ---- phase A: E^T accumulation (and s_in row sums) ----
            ew = G if shared else H * G
            # es: E^T [128, ew] cols 0:ew ; s_in row [1, ew] cols ew:2*ew
            # E = xh*ih + xh*il + xl*ih  (fp16 hi/lo split, fp32 psum accum)
            es = ps.tile([128, 2 * ew], F32, tag="es")
            def xsl(t):
                for xt, per, t0 in xtiles:
                    if t0 <= t < t0 + per:
                        o = (t - t0) * 2 * C
                        return xt[:, o : o + C], xt[:, o + C : o + 2 * C]
                raise AssertionError(t)

            for t in range(NT):
                xh, xl = xsl(t)
                first, last = t == 0, t == NT - 1
                if shared:
                    ih = ihl_sb[:, (2 * t) * G : (2 * t + 1) * G]
                    il = ihl_sb[:, (2 * t + 1) * G : (2 * t + 2) * G]
                    e = es[:, 0:G]
                    nc.tensor.matmul(e, xh, ih, start=first, stop=False)
                    nc.tensor.matmul(e, xh, il, start=False, stop=False)
                    nc.tensor.matmul(e, xl, ih, start=False, stop=last)
                else:
                    for h in range(H):
                        o = 2 * (h * NT + t) * G
                        ih = ihl_sb[:, o : o + G]
                        il = ihl_sb[:, o + G : o + 2 * G]
                        e = es[:, h * G : (h + 1) * G]
                        nc.tensor.matmul(e, xh, ih, start=first, stop=False)
                        nc.tensor.matmul(e, xh, il, start=False, stop=False)
                        nc.tensor.matmul(e, xl, ih, start=False, stop=last)
            # s_in: ones^T @ (ih + il) tiles -> [1, G] per head (if b_in != 0)
            if with_bias:
                for t in range(NT):
                    first, last = t == 0, t == NT - 1
                    if shared:
                        s = es[0:1, ew : ew + G]
                        o = 2 * t * G
                        nc.tensor.matmul(
                            s, ones16[:], ihl_sb[:, o : o + G],
                            start=first, stop=False,
                        )
                        nc.tensor.matmul(
                            s, ones16[:], ihl_sb[:, o + G : o + 2 * G],
                            start=False, stop=last,
                        )
                    else:
                        for h in range(H):
                            o = 2 * (h * NT + t) * G
                            s = es[0:1, ew + h * G : ew + (h + 1) * G]
                            nc.tensor.matmul(
                                s, ones16[:], ihl_sb[:, o : o + G],
                                start=first, stop=False,
                            )
                            nc.tensor.matmul(
                                s, ones16[:], ihl_sb[:, o + G : o + 2 * G],
                                start=False, stop=last,
                            )

            warm(3)
            e_sb = work.tile([128, ew], F32, tag="e_sb")
            nc.vector.tensor_copy(e_sb[:], es[:, 0:ew])
            s_sb = work.tile([1, ew], F32, tag="s_sb")
            if with_bias:
                nc.vector.tensor_copy(s_sb[:], es[0:1, ew : 2 * ew])

            # ---- phase B: specT packed [128, 128] ----
            # partition p = (h%2)*64 + dh ; free f = (h//2)*32 + g
            HG = H * G  # 256
            spec_ps = ps.tile([128, 128], F32, tag="es")  # reuse es slot (freed)
            for h in range(H):
                hlo, q = h % 2, h // 2
                reg = spec_ps[hlo * 64 : hlo * 64 + 64, q * G : (q + 1) * G]
                rhs = e_sb[:, 0:G] if shared else e_sb[:, h * G : (h + 1) * G]
                nc.tensor.matmul(
                    reg,
                    w_in_sb[:, h * DH : (h + 1) * DH],
                    rhs,
                    start=True,
                    stop=not with_bias,
                )
                if with_bias:
                    srhs = s_sb[0:1, 0:G] if shared else s_sb[0:1, h * G : (h + 1) * G]
                    nc.tensor.matmul(
                        reg,
                        b_in_sb[0:1, h * DH : (h + 1) * DH],
                        srhs,
                        start=False,
                        stop=True,
                    )

            spec_sb = work.tile([128, 128], F32, tag="spec_sb")
            nc.vector.tensor_copy(spec_sb[:], spec_ps[:])
            warm(3)

            # ---- phase C: LayerNorm over (G, DH) per head ----
            sq_sb = work.tile([128, 128], F32, tag="sq_sb")
            nc.vector.tensor_mul(sq_sb[:], spec_sb[:], spec_sb[:])
            red1 = work.tile([128, 4], F32, tag="red1")
            nc.vector.reduce_sum(
                out=red1[:],
                in_=spec_sb[:].rearrange("p (q g) -> p q g", g=G),
                axis=mybir.AxisListType.X,
            )
            red2 = work.tile([128, 4], F32, tag="red2")
            nc.vector.reduce_sum(
                out=red2[:],
                in_=sq_sb[:].rearrange("p (q g) -> p q g", g=G),
                axis=mybir.AxisListType.X,
            )
            # st[hlo, 0:4] = per-head means; st[hlo, 4:8] = mean-squares
            # (ones-style ind2 is pre-scaled to 1/(G*DH))
            st = ps.tile([2, 8], F32, tag="st")
            nc.tensor.matmul(st[:, 0:4], ind2[:], red1[:], start=True, stop=True)
            nc.tensor.matmul(st[:, 4:8], ind2[:], red2[:], start=True, stop=True)

            warm(3)
            mu_sb = work.tile([2, 4], F32, tag="mu_sb")
            nc.vector.tensor_copy(mu_sb[:], st[:, 0:4])
            var_sb = work.tile([2, 4], F32, tag="var_sb")
            nc.vector.tensor_mul(var_sb[:], mu_sb[:], mu_sb[:])
            nc.vector.tensor_sub(var_sb[:], st[:, 4:8], var_sb[:])
            std_sb = work.tile([2, 4], F32, tag="std_sb")
            nc.scalar.activation(
                out=std_sb[:],
                in_=var_sb[:],
                func=mybir.ActivationFunctionType.Sqrt,
                bias=eps_sb[:],
                scale=1.0,
            )
            rstd_sb = work.tile([2, 4], F32, tag="rstd_sb")
            nc.vector.reciprocal(rstd_sb[:], std_sb[:])

            # broadcast across partition halves via K=2 matmul -> [128, 4]
            # each; the 32x repeat along g happens via step-0 affine reads
            mr = ps.tile([128, 8], F32, tag="mr")
            nc.tensor.matmul(mr[:, 0:4], ind2t[:], mu_sb[:], start=True, stop=True)
            nc.tensor.matmul(mr[:, 4:8], ind2t[:], rstd_sb[:], start=True, stop=True)

            warm(4)

            def _bcast(ap):
                return bass.AP(
                    tensor=ap.tensor, offset=ap.offset, ap=[ap.ap[0], ap.ap[1], [0, G]]
                )

            aff = work.tile([128, 128], F32, tag="aff")
            aff3 = aff[:].rearrange("p (q g) -> p q g", g=G)
            spec3 = spec_sb[:].rearrange("p (q g) -> p q g", g=G)
            nc.vector.tensor_sub(aff3, spec3, _bcast(mr[:, 0:4]))
            nc.vector.tensor_mul(aff3, aff3, _bcast(mr[:, 4:8]))
            nc.vector.tensor_mul(aff[:], aff[:], lngt_sb[:])
            nc.vector.tensor_add(aff[:], aff[:], lnbt_sb[:])

            # ---- phase D: fused (mlp_w @ W_out) fold; mw2 stacks head pairs
            # so each matmul contracts K=128 = [dh(h=2q); dh(h=2q+1)] ----
            dw = 2 * C
            dflat_ps = ps.tile([G, (C if shared else H * C)], F32, tag="dflat")
            if shared:
                for q in range(4):
                    nc.tensor.matmul(
                        dflat_ps[0:G, 0:C],
                        aff[:, q * G : (q + 1) * G],
                        mw_sb[:, q * C : (q + 1) * C],
                        start=(q == 0),
                        stop=(q == 3),
                    )
            else:
                for h in range(H):
                    hlo, q = h % 2, h // 2
                    nc.tensor.matmul(
                        dflat_ps[0:G, h * C : (h + 1) * C],
                        aff[hlo * 64 : hlo * 64 + 64, q * G : (q + 1) * G],
                        mw_sb[hlo * 64 : hlo * 64 + 64, h * C : (h + 1) * C],
                        start=True,
                        stop=True,
                    )
            warm(3)
            # D split into fp16 hi/lo for the fast decode matmuls
            if shared:
                d32 = work.tile([G, C], F32, tag="d32")
                nc.vector.tensor_copy(d32[:], dflat_ps[0:G, 0:C])
            else:
                d32 = work.tile([128, dw], F32, tag="d32")
                for h in range(H):
                    nc.vector.tensor_copy(
                        d32[(h % 4) * G : (h % 4 + 1) * G,
                            (h // 4) * C : (h // 4 + 1) * C],
                        dflat_ps[0:G, h * C : (h + 1) * C],
                    )
            dh16 = work.tile(list(d32.shape), BF16, tag="dh16")
            nc.vector.tensor_copy(dh16[:], d32[:])
            dh32 = work.tile(list(d32.shape), F32, tag="dh32")
            nc.vector.tensor_copy(dh32[:], dh16[:])
            dl32 = work.tile(list(d32.shape), F32, tag="dl32")
            nc.vector.tensor_sub(dl32[:], d32[:], dh32[:])
            dl16 = work.tile(list(d32.shape), BF16, tag="dl16")
            nc.vector.tensor_copy(dl16[:], dl32[:])

            if shared:
                # single K=128 lhsT [Dh; Dh; Dl; 0] against rhs [Th; Tl; Th; Tl]
                dst_sb = work.tile([128, C], BF16, tag="dst")
                nc.vector.tensor_copy(dst_sb[0:G, :], dh16[:])
                nc.vector.tensor_copy(dst_sb[G : 2 * G, :], dh16[:])
                nc.vector.tensor_copy(dst_sb[64 : 64 + G, :], dl16[:])
                nc.vector.memset(dst_sb[96:128, :], 0.0)

            # ---- phase E: decode 16 chunks of 512 (one K=128 matmul each) ----
            for c in range(16):
                yps = psy.tile([C, 512], F32, tag="y")
                if shared:
                    rhs = invt_tiles[c // 4][:, (c % 4) * 512 : (c % 4 + 1) * 512]
                    nc.tensor.matmul(yps[:], dst_sb[:], rhs, start=True, stop=True)
                else:
                    col = (c // 4) * 2048 + (c % 4) * 512
                    for kb in range(2):
                        rhs_h = invt_sb[:, kb * 8192 + col :][:, 0:512]
                        rhs_l = invt_sb[:, 16384 + kb * 8192 + col :][:, 0:512]
                        lh = dh16[:, kb * C : (kb + 1) * C]
                        ll = dl16[:, kb * C : (kb + 1) * C]
                        nc.tensor.matmul(yps[:], lh, rhs_h, start=(kb == 0), stop=False)
                        nc.tensor.matmul(yps[:], lh, rhs_l, start=False, stop=False)
                        nc.tensor.matmul(yps[:], ll, rhs_h, start=False, stop=(kb == 1))
                # accumulate 4 chunks into one [128, 2048] store buffer;
                # evac alternates DVE / ACT so neither engine paces decode
                if c % 4 == 0:
                    y_sb = yout.tile([C, 2048], F32, tag="ysb")
                dst_ap = y_sb[:, (c % 4) * 512 : (c % 4 + 1) * 512]
                if c % 2 == 0:
                    nc.vector.tensor_scalar_add(dst_ap, yps[:], b_out_sb[:, 0:1])
                else:
                    nc.scalar.activation(
                        out=dst_ap,
                        in_=yps[:],
                        func=mybir.ActivationFunctionType.Identity,
                        bias=b_out_sb[:, 0:1],
                        scale=1.0,
                    )
                if c >= 12:
                    # tail group: store each chunk as it completes
                    eng = nc.sync if c % 2 == 0 else nc.scalar
                    eng.dma_start(
                        out=yt_d[:, c * 512 : (c + 1) * 512],
                        in_=y_sb[:, (c % 4) * 512 : (c % 4 + 1) * 512],
                    )
                elif c % 4 == 3:
                    eng = nc.sync if (c // 4) % 2 == 0 else nc.scalar
                    eng.dma_start(
                        out=yt_d[:, (c - 3) * 512 : (c + 1) * 512], in_=y_sb[:]
                    )

    nc.compile()
    return nc


def kernel(x, W_in, b_in, mlp_w, ln_g, ln_b, W_out, b_out, inv_in, inv_out):
    x = np.ascontiguousarray(x, dtype=np.float32)
    inv_in = np.asarray(inv_in, dtype=np.float32)
    inv_out = np.asarray(inv_out, dtype=np.float32)

    shared = all(
        np.array_equal(a[0], a[h]) for a in (inv_in, inv_out) for h in range(1, H)
    )
    with_bias = bool(np.any(np.asarray(b_in)))

    key = (shared, with_bias)
    if key not in _cache:
        _cache[key] = _build(shared, with_bias)
    nc = _cache[key]

    ln_gt = np.ascontiguousarray(np.asarray(ln_g, np.float32).T)  # [DH, G]
    lngt_p = np.tile(ln_gt, (2, 4))  # [128, 128]
    ln_bt = np.ascontiguousarray(np.asarray(ln_b, np.float32).T)
    lnbt_p = np.tile(ln_bt, (2, 4))

    def split16(a):
        hi = a.astype(np.float16)
        lo = (a - hi.astype(np.float32)).astype(np.float16)
        return hi, lo

    import ml_dtypes

    def splitbf(a):
        hi = a.astype(ml_dtypes.bfloat16)
        lo = (a - hi.astype(np.float32)).astype(ml_dtypes.bfloat16)
        return hi, lo

    def shuffle_hl(a, last):
        """[..., S*128, last] fp32 -> [128, ... , S, 2, last] fp16 hi/lo packed,
        flattened to [128, -1] with partition = row % 128."""
        hi, lo = split16(a)
        st = np.stack((hi, lo), axis=-2)  # [..., S*128, 2, last]
        lead = st.shape[:-3]
        st = st.reshape(*lead, -1, 128, 2, last)  # [..., S, 128, 2, last]
        st = np.moveaxis(st, -3, 0)  # [128, ..., S, 2, last]
        return np.ascontiguousarray(st.reshape(128, -1))

    common = {
        "w_in": np.ascontiguousarray(W_in, np.float32),
        "mw": _mw_pack(mlp_w, W_out, shared),
        "ind2t": _ind2t(),
        "lngt": np.ascontiguousarray(lngt_p, np.float32),
        "lnbt": np.ascontiguousarray(lnbt_p, np.float32),
        "b_in": np.ascontiguousarray(np.asarray(b_in, np.float32).reshape(1, HD)),
        "b_out": np.ascontiguousarray(np.asarray(b_out, np.float32).reshape(C, 1)),
    }
    if shared:
        common["ihl"] = shuffle_hl(inv_in[0], G)  # [128, NT*2*G]
        th, tl = splitbf(np.ascontiguousarray(inv_out[0].T))  # [G, N] each
        # stacked [4 blocks, 64, 2048]: rows = [Th; Tl]
        st = np.empty((4, 64, 2048), ml_dtypes.bfloat16)
        for blk in range(4):
            sl = slice(blk * 2048, (blk + 1) * 2048)
            st[blk, 0:G] = th[:, sl]
            st[blk, G:64] = tl[:, sl]
        common["invtst"] = np.ascontiguousarray(st.reshape(256, 2048))
    else:
        # ihl layout [128, (h, s, j, g)]: shuffle per head then concat on free
        per_h = [shuffle_hl(inv_in[h], G) for h in range(H)]
        common["ihl"] = np.ascontiguousarray(np.concatenate(per_h, axis=1))
        invt_arr = np.ascontiguousarray(inv_out.transpose(0, 2, 1).reshape(H * G, N))
        hi, lo = splitbf(invt_arr)
        common["invth"] = np.ascontiguousarray(hi)
        common["invtl"] = np.ascontiguousarray(lo)

    in_maps = [dict(common, xhl=shuffle_hl(x[i], C)) for i in range(B)]

    trace = bool(os.environ.get("SPECMIX_TRACE"))
    res = run_bass_kernel_spmd(nc, in_maps, list(range(B)), trace=trace)
    if trace:
        global LAST_EXEC_NS
        LAST_EXEC_NS = res.exec_time_ns
    out = np.empty((B, N, C), np.float32)
    for i in range(B):
        out[i] = res.results[i]["yT"].T
    return out


if __name__ == "__main__":
    rng = np.random.default_rng(0)
    ins = {
        "x": rng.standard_normal((B, N, C), np.float32),
        "W_in": rng.standard_normal((C, HD), np.float32) * 0.02,
        "b_in": np.zeros((HD,), np.float32),
        "mlp_w": rng.standard_normal((DH, DH), np.float32) * 0.02,
        "ln_g": np.ones((G, DH), np.float32),
        "ln_b": np.zeros((G, DH), np.float32),
        "W_out": rng.standard_normal((HD, C), np.float32) * 0.02,
        "b_out": np.zeros((C,), np.float32),
        "inv_in": np.broadcast_to(rng.standard_normal((1, N, G), np.float32), (H, N, G)).copy(),
        "inv_out": np.broadcast_to(rng.standard_normal((1, N, G), np.float32), (H, N, G)).copy(),
    }
    y = kernel(**ins)
    print("out", y.shape, y.dtype, np.abs(y).max())


# revision 65
# speedup vs baseline: 1.1452x; 1.1452x over previous
"""Trainium2 Bass kernel for the Calibrated Spectral Mixer (two-domain).

Math (per batch b):
    fx   = x @ W_in + b_in                      [N, H*DH]
    spec_h = inv_in_h^T @ fx_h                  [G, DH]   (contract over N)
    spec = LN_{(G,DH)}(spec) * ln_g + ln_b
    spec2_h = spec_h @ mlp_w
    out_h = inv_out_h @ spec2_h                 [N, DH]
    y    = concat_h(out_h) @ W_out + b_out      [N, C]

Device algebra (folded):
    E^T   = x^T-contraction:  E^T[c,g] = sum_n x[n,c] inv_in[n,g]
    specT_h[dh,g] = W_in_h^T @ E^T (+ b_in_h ⊗ s_in, s_in = sum_n inv_in[:,g])
    LN applied on packed specT, then spec2T_h = mlp_w^T @ affineT_h
    D[g,c] = sum_h spec2_h @ W_out_h    (heads summed when basis shared)
    yT[c,n] = D^T-contraction with inv_out^T  (+ b_out per-partition)

Sharding: data-parallel over B; core i computes batch i. No collectives.
If inv_in / inv_out are head-broadcast (they are, for the reference input
generator), only one head's basis is shipped/read (1 MB instead of 8 MB each).
"""

import os
import sys

sys.path.insert(0, "/opt/trn_rl_repo")

import numpy as np

import concourse.bass as bass
import concourse.tile as tile
from concourse import bacc, mybir
from concourse.bass_utils import run_bass_kernel_spmd

B, N, C, H, DH, G = 8, 8192, 128, 8, 64, 32
HD = H * DH  # 512
NT = N // 128  # 64 n-tiles of 128 rows
LN_EPS = 1e-5
F32 = mybir.dt.float32
F16 = mybir.dt.float16
BF16 = mybir.dt.bfloat16

_cache: dict = {}
LAST_EXEC_NS = None


def _ind2t():
    a = np.zeros((2, 128), np.float32)
    a[0, 0:64] = 1.0
    a[1, 64:128] = 1.0
    return a


def _mw_pack(mlp_w, W_out, shared):
    mw = np.einsum(
        "io,hoc->hic",
        np.asarray(mlp_w, np.float32),
        np.asarray(W_out, np.float32).reshape(H, DH, C),
    )  # [H, DH, C]
    if shared:
        out = np.empty((128, 4 * C), np.float32)
        for q in range(4):
            out[0:DH, q * C : (q + 1) * C] = mw[2 * q]
            out[DH:128, q * C : (q + 1) * C] = mw[2 * q + 1]
    else:
        out = np.zeros((128, H * C), np.float32)
        for h in range(H):
            b = (h % 2) * DH
            out[b : b + DH, h * C : (h + 1) * C] = mw[h]
    return np.ascontiguousarray(out)


def _build(shared: bool, with_bias: bool):
    """Build + compile the per-core Bass program.

    shared=True: inv bases identical across heads (1-head basis shipped).
    with_bias=False: b_in is all zeros -> skip the s_in / correction matmuls.
    """
    nc = bacc.Bacc("TRN2", target_bir_lowering=False, debug=False)

    # x and the bases are shipped as fp16 hi/lo pairs (same total bytes as
    # fp32); fp16 hi+lo carries ~22 mantissa bits, and the 3-term product
    # xh*ih + xh*il + xl*ih runs on the PE at 1 cy/col instead of fp32's
    # two-pass half-speed emulation.
    # xhl: host-pre-shuffled [p, (t, hi/lo, c)] so every DMA is 2D with long
    # contiguous runs per partition (cheap HWDGE descriptor generation)
    xhl_d = nc.dram_tensor("xhl", [128, NT * 2 * C], F16, kind="ExternalInput")
    w_in_d = nc.dram_tensor("w_in", [C, HD], F32, kind="ExternalInput")
    mw_d = nc.dram_tensor("mw", [128, (4 if shared else 8) * C], F32, kind="ExternalInput")
    ind2t_d = nc.dram_tensor("ind2t", [2, 128], F32, kind="ExternalInput")
    lngt_d = nc.dram_tensor("lngt", [128, 128], F32, kind="ExternalInput")
    lnbt_d = nc.dram_tensor("lnbt", [128, 128], F32, kind="ExternalInput")
    b_in_d = nc.dram_tensor("b_in", [1, HD], F32, kind="ExternalInput")
    b_out_d = nc.dram_tensor("b_out", [C, 1], F32, kind="ExternalInput")
    if shared:
        # ihl: host-pre-shuffled [p, (s, hi/lo, g)]
        ihl_d = nc.dram_tensor("ihl", [128, NT * 2 * G], F16, kind="ExternalInput")
        # stacked decode basis: 4 blocks of [Th; Tl] (duplicated on device)
        invtst_d = nc.dram_tensor("invtst", [4 * 64, 2048], BF16, kind="ExternalInput")
    else:
        # ihl: host-pre-shuffled [p, (h, s, hi/lo, g)]
        ihl_d = nc.dram_tensor("ihl", [128, H * NT * 2 * G], F16, kind="ExternalInput")
        invth_d = nc.dram_tensor("invth", [H * G, N], BF16, kind="ExternalInput")
        invtl_d = nc.dram_tensor("invtl", [H * G, N], BF16, kind="ExternalInput")
    yt_d = nc.dram_tensor("yT", [C, N], F32, kind="ExternalOutput")

    with tile.TileContext(nc) as tc:
        with (
            tc.tile_pool(name="res", bufs=1) as res,
            tc.tile_pool(name="work", bufs=1) as work,
            tc.tile_pool(name="yout", bufs=3) as yout,
            tc.tile_pool(name="ps", bufs=1, space="PSUM") as ps,
            tc.tile_pool(name="psy", bufs=3 if shared else 2, space="PSUM") as psy,
        ):
            # ---- resident loads ----
            # All DMAs are plain 2D copies (host pre-shuffled the layouts).
            # Issue is spread over both HWDGE rings (sync + scalar) and SWDGE
            # (gpsimd) so descriptor generation isn't serialized on one
            # sequencer; per-engine program order = need order.
            iw = NT * 2 * G if shared else H * NT * 2 * G
            ihl_sb = res.tile([128, iw], F16, tag="ihl")
            if shared:
                for k in range(4):
                    eng = nc.sync if k % 2 == 0 else nc.scalar
                    eng.dma_start(
                        out=ihl_sb[:, k * 1024 : (k + 1) * 1024],
                        in_=ihl_d[:, k * 1024 : (k + 1) * 1024],
                    )
            else:
                half = iw // 2
                nc.sync.dma_start(out=ihl_sb[:, 0:half], in_=ihl_d[:, 0:half])
                nc.scalar.dma_start(out=ihl_sb[:, half:iw], in_=ihl_d[:, half:iw])

            # x: slab 0 split into 4 fine tiles (encode can start after
            # ~256 KB lands); slabs 1-3 are 1 MB tiles with 2 DMAs each
            xtiles = []   # (tile, tiles_per, t0)
            for j in range(4):
                xt = res.tile([128, 4 * 2 * C], F16, tag=f"x0{j}")
                eng = nc.sync if j % 2 == 0 else nc.scalar
                eng.dma_start(
                    out=xt[:], in_=xhl_d[:, j * 1024 : (j + 1) * 1024]
                )
                xtiles.append((xt, 4, 4 * j))
            for i in range(1, 4):
                xt = res.tile([128, 16 * 2 * C], F16, tag=f"x{i}")
                eng = nc.sync if i % 2 == 0 else nc.scalar
                eng.dma_start(
                    out=xt[:, 0:2048], in_=xhl_d[:, i * 4096 : i * 4096 + 2048]
                )
                eng.dma_start(
                    out=xt[:, 2048:4096],
                    in_=xhl_d[:, i * 4096 + 2048 : (i + 1) * 4096],
                )
                xtiles.append((xt, 16, 16 * i))

            if shared:
                pass
            else:
                # invT hi/lo [H*G, N] -> [128, (j, kb, blk, f)]: partition = hg%128
                invt_sb = res.tile([128, 2 * 2 * 4 * 2048], BF16, tag="invt")
                for j, src in enumerate((invth_d, invtl_d)):
                    for kb in range(2):
                        for blk in range(4):
                            nc.gpsimd.dma_start(
                                out=invt_sb[:, j * 16384 + kb * 8192 + blk * 2048 :][:, 0:2048],
                                in_=src[kb * 128 : (kb + 1) * 128,
                                        blk * 2048 : (blk + 1) * 2048],
                            )

            w_in_sb = res.tile([C, HD], F32, tag="w_in")
            nc.gpsimd.dma_start(out=w_in_sb[:], in_=w_in_d[:])
            # mw = mlp_w @ W_out_h, host-precomputed (head-pair stacked)
            mw_sb = res.tile([128, (4 if shared else 8) * C], F32, tag="mw")
            nc.gpsimd.dma_start(out=mw_sb[:], in_=mw_d[:])
            lngt_sb = res.tile([128, 128], F32, tag="lngt")
            nc.gpsimd.dma_start(out=lngt_sb[:], in_=lngt_d[:])
            lnbt_sb = res.tile([128, 128], F32, tag="lnbt")
            nc.gpsimd.dma_start(out=lnbt_sb[:], in_=lnbt_d[:])
            b_in_sb = res.tile([1, HD], F32, tag="b_in")
            nc.gpsimd.dma_start(out=b_in_sb[:], in_=b_in_d[:])
            b_out_sb = res.tile([C, 1], F32, tag="b_out")
            nc.gpsimd.dma_start(out=b_out_sb[:], in_=b_out_d[:])

            if shared:
                invt_half = []
                for v in range(2):
                    th = res.tile([128, 2048], BF16, tag=f"invth{v}")
                    nc.gpsimd.dma_start(
                        out=th[:], in_=invtst_d[v * 128 : (v + 1) * 128, :]
                    )
                    invt_half.append(th)
                # duplicate [Th;Tl] -> [Th;Tl;Th;Tl] with idle-DVE copies
                invt_tiles = []
                for u in range(4):
                    tt = res.tile([128, 2048], BF16, tag=f"invt{u}")
                    src = invt_half[u // 2][(u % 2) * 64 : (u % 2) * 64 + 64, :]
                    nc.vector.tensor_copy(tt[0:64, :], src)
                    nc.vector.tensor_copy(tt[64:128, :], src)
                    invt_tiles.append(tt)

            ones16 = res.tile([128, 1], F16, tag="ones16")
            nc.vector.memset(ones16[:], 1.0)
            # half-indicators, pre-scaled so the stats matmul yields means
            ind2 = res.tile([128, 2], F32, tag="ind2")
            nc.vector.memset(ind2[:], 0.0)
            nc.vector.memset(ind2[0:64, 0:1], 1.0 / (G * DH))
            nc.vector.memset(ind2[64:128, 1:2], 1.0 / (G * DH))
            ind2t = res.tile([2, 128], F32, tag="ind2t")
            nc.gpsimd.dma_start(out=ind2t[:], in_=ind2t_d[:])
            eps_sb = res.tile([2, 1], F32, tag="eps")
            nc.vector.memset(eps_sb[:], LN_EPS)

            # PE warm-up: the HAM clock-gate only ramps to 2.4 GHz after
            # ~3.4 us of sustained activity and decays when PE idles. Dummy
            # matmuls (result discarded) run during DMA stalls and mid-phase
            # gaps so the real matmuls execute warm.
            warm_ps = ps.tile([1, 512], F32, tag="warm")

            def warm(n):
                for _ in range(n):
                    nc.tensor.matmul(
                        warm_ps[0:1, 0:512],
                        ones16[:],
                        ihl_sb[:, 0:512],
                        start=True,
                        stop=True,
                    )

            warm(12)

            # ---- phase A: E^T accumulation (and s_in row sums) ----
            ew = G if shared else H * G
            # es: E^T [128, ew] cols 0:ew ; s_in row [1, ew] cols ew:2*ew
            # E = xh*ih + xh*il + xl*ih  (fp16 hi/lo split, fp32 psum accum)
            es = ps.tile([128, 2 * ew], F32, tag="es")
            def xsl(t):
                for xt, per, t0 in xtiles:
                    if t0 <= t < t0 + per:
                        o = (t - t0) * 2 * C
                        return xt[:, o : o + C], xt[:, o + C : o + 2 * C]
                raise AssertionError(t)

            for t in range(NT):
                xh, xl = xsl(t)
                first, last = t == 0, t == NT - 1
                if shared:
                    ih = ihl_sb[:, (2 * t) * G : (2 * t + 1) * G]
                    il = ihl_sb[:, (2 * t + 1) * G : (2 * t + 2) * G]
                    e = es[:, 0:G]
                    nc.tensor.matmul(e, xh, ih, start=first, stop=False)
                    nc.tensor.matmul(e, xh, il, start=False, stop=False)
                    nc.tensor.matmul(e, xl, ih, start=False, stop=last)
                else:
                    for h in range(H):
                        o = 2 * (h * NT + t) * G
                        ih = ihl_sb[:, o : o + G]
                        il = ihl_sb[:, o + G : o + 2 * G]
                        e = es[:, h * G : (h + 1) * G]
                        nc.tensor.matmul(e, xh, ih, start=first, stop=False)
                        nc.tensor.matmul(e, xh, il, start=False, stop=False)
                        nc.tensor.matmul(e, xl, ih, start=False, stop=last)
            # s_in: ones^T @ (ih + il) tiles -> [1, G] per head (if b_in != 0)
            if with_bias:
                for t in range(NT):
                    first, last = t == 0, t == NT - 1
                    if shared:
                        s = es[0:1, ew : ew + G]
                        o = 2 * t * G
                        nc.tensor.matmul(
                            s, ones16[:], ihl_sb[:, o : o + G],
                            start=first, stop=False,
                        )
                        nc.tensor.matmul(
                            s, ones16[:], ihl_sb[:, o + G : o + 2 * G],
                            start=False, stop=last,
                        )
                    else:
                        for h in range(H):
                            o = 2 * (h * NT + t) * G
                            s = es[0:1, ew + h * G : ew + (h + 1) * G]
                            nc.tensor.matmul(
                                s, ones16[:], ihl_sb[:, o : o + G],
                                start=first, stop=False,
                            )
                            nc.tensor.matmul(
                                s, ones16[:], ihl_sb[:, o + G : o + 2 * G],
                                start=False, stop=last,
                            )

            warm(3)
            e_sb = work.tile([128, ew], F32, tag="e_sb")
            nc.vector.tensor_copy(e_sb[:], es[:, 0:ew])
            s_sb = work.tile([1, ew], F32, tag="s_sb")
            if with_bias:
                nc.vector.tensor_copy(s_sb[:], es[0:1, ew : 2 * ew])

            # ---- phase B: specT packed [128, 128] ----
            # partition p = (h%2)*64 + dh ; free f = (h//2)*32 + g
            HG = H * G  # 256
            spec_ps = ps.tile([128, 128], F32, tag="es")  # reuse es slot (freed)
            for h in range(H):
                hlo, q = h % 2, h // 2
                reg = spec_ps[hlo * 64 : hlo * 64 + 64, q * G : (q + 1) * G]
                rhs = e_sb[:, 0:G] if shared else e_sb[:, h * G : (h + 1) * G]
                nc.tensor.matmul(
                    reg,
                    w_in_sb[:, h * DH : (h + 1) * DH],
                    rhs,
                    start=True,
                    stop=not with_bias,
                )
                if with_bias:
                    srhs = s_sb[0:1, 0:G] if shared else s_sb[0:1, h * G : (h + 1) * G]
                    nc.tensor.matmul(
                        reg,
                        b_in_sb[0:1, h * DH : (h + 1) * DH],
                        srhs,
                        start=False,
                        stop=True,
                    )

            spec_sb = work.tile([128, 128], F32, tag="spec_sb")
            nc.vector.tensor_copy(spec_sb[:], spec_ps[:])
            warm(3)

            # ---- phase C: LayerNorm over (G, DH) per head ----
            sq_sb = work.tile([128, 128], F32, tag="sq_sb")
            nc.vector.tensor_mul(sq_sb[:], spec_sb[:], spec_sb[:])
            red1 = work.tile([128, 4], F32, tag="red1")
            nc.vector.reduce_sum(
                out=red1[:],
                in_=spec_sb[:].rearrange("p (q g) -> p q g", g=G),
                axis=mybir.AxisListType.X,
            )
            red2 = work.tile([128, 4], F32, tag="red2")
            nc.vector.reduce_sum(
                out=red2[:],
                in_=sq_sb[:].rearrange("p (q g) -> p q g", g=G),
                axis=mybir.AxisListType.X,
            )
            # st[hlo, 0:4] = per-head means; st[hlo, 4:8] = mean-squares
            # (ones-style ind2 is pre-scaled to 1/(G*DH))
            st = ps.tile([2, 8], F32, tag="st")
            nc.tensor.matmul(st[:, 0:4], ind2[:], red1[:], start=True, stop=True)
            nc.tensor.matmul(st[:, 4:8], ind2[:], red2[:], start=True, stop=True)

            warm(3)
            mu_sb = work.tile([2, 4], F32, tag="mu_sb")
            nc.vector.tensor_copy(mu_sb[:], st[:, 0:4])
            var_sb = work.tile([2, 4], F32, tag="var_sb")
            nc.vector.tensor_mul(var_sb[:], mu_sb[:], mu_sb[:])
            nc.vector.tensor_sub(var_sb[:], st[:, 4:8], var_sb[:])
            std_sb = work.tile([2, 4], F32, tag="std_sb")
            nc.scalar.activation(
                out=std_sb[:],
                in_=var_sb[:],
                func=mybir.ActivationFunctionType.Sqrt,
                bias=eps_sb[:],
                scale=1.0,
            )
            rstd_sb = work.tile([2, 4], F32, tag="rstd_sb")
            nc.vector.reciprocal(rstd_sb[:], std_sb[:])

            # broadcast across partition halves via K=2 matmul -> [128, 4]
            # each; the 32x repeat along g happens via step-0 affine reads
            mr = ps.tile([128, 8], F32, tag="mr")
            nc.tensor.matmul(mr[:, 0:4], ind2t[:], mu_sb[:], start=True, stop=True)
            nc.tensor.matmul(mr[:, 4:8], ind2t[:], rstd_sb[:], start=True, stop=True)

            warm(4)

            def _bcast(ap):
                return bass.AP(
                    tensor=ap.tensor, offset=ap.offset, ap=[ap.ap[0], ap.ap[1], [0, G]]
                )

            aff = work.tile([128, 128], F32, tag="aff")
            aff3 = aff[:].rearrange("p (q g) -> p q g", g=G)
            spec3 = spec_sb[:].rearrange("p (q g) -> p q g", g=G)
            nc.vector.tensor_sub(aff3, spec3, _bcast(mr[:, 0:4]))
            nc.vector.tensor_mul(aff3, aff3, _bcast(mr[:, 4:8]))
            nc.vector.tensor_mul(aff[:], aff[:], lngt_sb[:])
            nc.vector.tensor_add(aff[:], aff[:], lnbt_sb[:])

            # ---- phase D: fused (mlp_w @ W_out) fold; mw2 stacks head pairs
            # so each matmul contracts K=128 = [dh(h=2q); dh(h=2q+1)] ----
            dw = 2 * C
            dflat_ps = ps.tile([G, (C if shared else H * C)], F32, tag="dflat")
            if shared:
                for q in range(4):
                    nc.tensor.matmul(
                        dflat_ps[0:G, 0:C],
                        aff[:, q * G : (q + 1) * G],
                        mw_sb[:, q * C : (q + 1) * C],
                        start=(q == 0),
                        stop=(q == 3),
                    )
            else:
                for h in range(H):
                    hlo, q = h % 2, h // 2
                    nc.tensor.matmul(
                        dflat_ps[0:G, h * C : (h + 1) * C],
                        aff[hlo * 64 : hlo * 64 + 64, q * G : (q + 1) * G],
                        mw_sb[hlo * 64 : hlo * 64 + 64, h * C : (h + 1) * C],
                        start=True,
                        stop=True,
                    )
            warm(3)
            # D split into fp16 hi/lo for the fast decode matmuls
            if shared:
                d32 = work.tile([G, C], F32, tag="d32")
                nc.vector.tensor_copy(d32[:], dflat_ps[0:G, 0:C])
            else:
                d32 = work.tile([128, dw], F32, tag="d32")
                for h in range(H):
                    nc.vector.tensor_copy(
                        d32[(h % 4) * G : (h % 4 + 1) * G,
                            (h // 4) * C : (h // 4 + 1) * C],
                        dflat_ps[0:G, h * C : (h + 1) * C],
                    )
            dh16 = work.tile(list(d32.shape), BF16, tag="dh16")
            nc.vector.tensor_copy(dh16[:], d32[:])
            dh32 = work.tile(list(d32.shape), F32, tag="dh32")
            nc.vector.tensor_copy(dh32[:], dh16[:])
            dl32 = work.tile(list(d32.shape), F32, tag="dl32")
            nc.vector.tensor_sub(dl32[:], d32[:], dh32[:])
            dl16 = work.tile(list(d32.shape), BF16, tag="dl16")
            nc.vector.tensor_copy(dl16[:], dl32[:])

            if shared:
                # single K=128 lhsT [Dh; Dh; Dl; 0] against rhs [Th; Tl; Th; Tl]
                dst_sb = work.tile([128, C], BF16, tag="dst")
                nc.vector.tensor_copy(dst_sb[0:G, :], dh16[:])
                nc.vector.tensor_copy(dst_sb[G : 2 * G, :], dh16[:])
                nc.vector.tensor_copy(dst_sb[64 : 64 + G, :], dl16[:])
                nc.vector.memset(dst_sb[96:128, :], 0.0)

            # ---- phase E: decode 16 chunks of 512 (one K=128 matmul each) ----
            for c in range(16):
                yps = psy.tile([C, 512], F32, tag="y")
                if shared:
                    rhs = invt_tiles[c // 4][:, (c % 4) * 512 : (c % 4 + 1) * 512]
                    nc.tensor.matmul(yps[:], dst_sb[:], rhs, start=True, stop=True)
                else:
                    col = (c // 4) * 2048 + (c % 4) * 512
                    for kb in range(2):
                        rhs_h = invt_sb[:, kb * 8192 + col :][:, 0:512]
                        rhs_l = invt_sb[:, 16384 + kb * 8192 + col :][:, 0:512]
                        lh = dh16[:, kb * C : (kb + 1) * C]
                        ll = dl16[:, kb * C : (kb + 1) * C]
                        nc.tensor.matmul(yps[:], lh, rhs_h, start=(kb == 0), stop=False)
                        nc.tensor.matmul(yps[:], lh, rhs_l, start=False, stop=False)
                        nc.tensor.matmul(yps[:], ll, rhs_h, start=False, stop=(kb == 1))
                # accumulate 4 chunks into one [128, 2048] store buffer;
                # evac alternates DVE / ACT so neither engine paces decode
                if c % 4 == 0:
                    y_sb = yout.tile([C, 2048], F32, tag="ysb")
                dst_ap = y_sb[:, (c % 4) * 512 : (c % 4 + 1) * 512]
                if c % 2 == 0:
                    nc.vector.tensor_scalar_add(dst_ap, yps[:], b_out_sb[:, 0:1])
                else:
                    nc.scalar.activation(
                        out=dst_ap,
                        in_=yps[:],
                        func=mybir.ActivationFunctionType.Identity,
                        bias=b_out_sb[:, 0:1],
                        scale=1.0,
                    )
                if c % 4 == 3:
                    eng = nc.sync if (c // 4) % 2 == 0 else nc.scalar
                    eng.dma_start(
                        out=yt_d[:, (c - 3) * 512 : (c + 1) * 512], in_=y_sb[:]
                    )

    nc.compile()
    return nc


def kernel(x, W_in, b_in, mlp_w, ln_g, ln_b, W_out, b_out, inv_in, inv_out):
    x = np.ascontiguousarray(x, dtype=np.float32)
    inv_in = np.asarray(inv_in, dtype=np.float32)
    inv_out = np.asarray(inv_out, dtype=np.float32)

    shared = all(
        np.array_equal(a[0], a[h]) for a in (inv_in, inv_out) for h in range(1, H)
    )
    with_bias = bool(np.any(np.asarray(b_in)))

    key = (shared, with_bias)
    if key not in _cache:
        _cache[key] = _build(shared, with_bias)
    nc = _cache[key]

    ln_gt = np.ascontiguousarray(np.asarray(ln_g, np.float32).T)  # [DH, G]
    lngt_p = np.tile(ln_gt, (2, 4))  # [128, 128]
    ln_bt = np.ascontiguousarray(np.asarray(ln_b, np.float32).T)
    lnbt_p = np.tile(ln_bt, (2, 4))

    def split16(a):
        hi = a.astype(np.float16)
        lo = (a - hi.astype(np.float32)).astype(np.float16)
        return hi, lo

    import ml_dtypes

    def splitbf(a):
        hi = a.astype(ml_dtypes.bfloat16)
        lo = (a - hi.astype(np.float32)).astype(ml_dtypes.bfloat16)
        return hi, lo

    def shuffle_hl(a, last):
        """[..., S*128, last] fp32 -> [128, ... , S, 2, last] fp16 hi/lo packed,
        flattened to [128, -1] with partition = row % 128."""
        hi, lo = split16(a)
        st = np.stack((hi, lo), axis=-2)  # [..., S*128, 2, last]
        lead = st.shape[:-3]
        st = st.reshape(*lead, -1, 128, 2, last)  # [..., S, 128, 2, last]
        st = np.moveaxis(st, -3, 0)  # [128, ..., S, 2, last]
        return np.ascontiguousarray(st.reshape(128, -1))

    common = {
        "w_in": np.ascontiguousarray(W_in, np.float32),
        "mw": _mw_pack(mlp_w, W_out, shared),
        "ind2t": _ind2t(),
        "lngt": np.ascontiguousarray(lngt_p, np.float32),
        "lnbt": np.ascontiguousarray(lnbt_p, np.float32),
        "b_in": np.ascontiguousarray(np.asarray(b_in, np.float32).reshape(1, HD)),
        "b_out": np.ascontiguousarray(np.asarray(b_out, np.float32).reshape(C, 1)),
    }
    if shared:
        common["ihl"] = shuffle_hl(inv_in[0], G)  # [128, NT*2*G]
        th, tl = splitbf(np.ascontiguousarray(inv_out[0].T))  # [G, N] each
        # stacked [4 blocks, 64, 2048]: rows = [Th; Tl]
        st = np.empty((4, 64, 2048), ml_dtypes.bfloat16)
        for blk in range(4):
            sl = slice(blk * 2048, (blk + 1) * 2048)
            st[blk, 0:G] = th[:, sl]
            st[blk, G:64] = tl[:, sl]
        common["invtst"] = np.ascontiguousarray(st.reshape(256, 2048))
    else:
        # ihl layout [128, (h, s, j, g)]: shuffle per head then concat on free
        per_h = [shuffle_hl(inv_in[h], G) for h in range(H)]
        common["ihl"] = np.ascontiguousarray(np.concatenate(per_h, axis=1))
        invt_arr = np.ascontiguousarray(inv_out.transpose(0, 2, 1).reshape(H * G, N))
        hi, lo = splitbf(invt_arr)
        common["invth"] = np.ascontiguousarray(hi)
        common["invtl"] = np.ascontiguousarray(lo)

    in_maps = [dict(common, xhl=shuffle_hl(x[i], C)) for i in range(B)]

    trace = bool(os.environ.get("SPECMIX_TRACE"))
    res = run_bass_kernel_spmd(nc, in_maps, list(range(B)), trace=trace)
    if trace:
        global LAST_EXEC_NS
        LAST_EXEC_NS = res.exec_time_ns
    out = np.empty((B, N, C), np.float32)
    for i in range(B):
        out[i] = res.results[i]["yT"].T
    return out


if __name__ == "__main__":
    rng = np.random.default_rng(0)
    ins = {
        "x": rng.standard_normal((B, N, C), np.float32),
        "W_in": rng.standard_normal((C, HD), np.float32) * 0.02,
        "b_in": np.zeros((HD,), np.float32),
        "mlp_w": rng.standard_normal((DH, DH), np.float32) * 0.02,
        "ln_g": np.ones((G, DH), np.float32),
        "ln_b": np.zeros((G, DH), np.float32),
        "W_out": rng.standard_normal((HD, C), np.float32) * 0.02,
        "b_out": np.zeros((C,), np.float32),
        "inv_in": np.broadcast_to(rng.standard_normal((1, N, G), np.float32), (H, N, G)).copy(),
        "inv_out": np.broadcast_to(rng.standard_normal((1, N, G), np.float32), (H, N, G)).copy(),
    }
    y = kernel(**ins)
    print("out", y.shape, y.dtype, np.abs(y).max())
